# revision 1
# baseline (speedup 1.0000x reference)
"""MAGNO encoder kernel distributed across 8 Trainium2 NeuronCores.

Sharding: device d in [0,8) handles batch b = d//4 and latent-row quarter
q = d%4 (rows [4096q, 4096(q+1))). row_idx is sorted, so each (batch,
scale, quarter)'s edges are a contiguous range — the host finds the
boundaries, pads each range to a common cap, and ships local row ids.
Every edge for a given latent row lands on exactly one device, so each
device's segment sums/counts are already global: it applies the mean and
softmax scale-weighting on device and returns its final disjoint
[4096, COUT] output slice. The host just concatenates the 8 slices.

Two device stages (separate pmaps): gather+MLP, then segment-sum+finish.
"""

import numpy as np
import jax
import jax.numpy as jnp
from functools import partial

B, N, M, S, E = 2, 100000, 16384, 3, 262144
CD, CIN, COUT, HID = 2, 3, 32, 64
NDEV = 8
MQ = M // 4  # latent rows per device


def _kernel_mlp(a, W1, b1, W2, b2, W3, b3):
    h = jax.nn.gelu(a @ W1.T + b1)
    h = jax.nn.gelu(h @ W2.T + b2)
    return h @ W3.T + b3


@partial(jax.pmap, axis_name="d",
         in_axes=(0, 0, 0, 0, None, None, None, None, None, None, None,
                  None, None))
def _stage_a(xb, pnb_raw, nbrs, rows_g, lat,
             W_lift, b_lift, W1, b1, W2, b2, W3, b3):
    pn = pnb_raw @ W_lift.T + b_lift                       # [N, COUT]
    ks = []
    for i in range(S):
        nbr, row = nbrs[i], rows_g[i]
        a = jnp.concatenate([xb[nbr], lat[row]], axis=-1)  # [ECAP, 2CD]
        k = _kernel_mlp(a, W1, b1, W2, b2, W3, b3)         # [ECAP, COUT]
        ks.append(k * pn[nbr])
    return jnp.stack(ks)                                   # [S, ECAP, COUT]


@partial(jax.pmap, axis_name="d", in_axes=(0, 0, 0))
def _stage_b(ks, rows_l, wcnt_q):
    # rows_l in [0, MQ] (MQ = padding sentinel)
    # wcnt_q: [S, MQ] = softmax_weight / max(count, 1), host-precomputed,
    # so the segment mean + scale weighting collapse to one multiply.
    acc = jnp.zeros((MQ, COUT), jnp.float32)
    for i in range(S):
        s = jax.ops.segment_sum(ks[i], rows_l[i], num_segments=MQ + 1,
                                indices_are_sorted=True)[:MQ]
        acc = acc + s * wcnt_q[i][:, None]
    return acc                                             # [MQ, COUT]


def _softmax_weights(lat, Ws1, bs1, Ws2, bs2):
    h = np.maximum(lat @ Ws1.T + bs1, 0.0) @ Ws2.T + bs2   # [M, S]
    h -= h.max(axis=-1, keepdims=True)
    e = np.exp(h)
    return e / e.sum(axis=-1, keepdims=True)               # [M, S]


def _numpy_fallback(x_coord, pndata, lat, nbr, row, W_lift, b_lift,
                    W1, b1, W2, b2, W3, b3, sw):
    def gelu(x):
        return 0.5 * x * (1.0 + np.tanh(np.sqrt(2 / np.pi) * (x + 0.044715 * x ** 3)))
    out = np.zeros((B, M, COUT), np.float32)
    for b in range(B):
        pn = pndata[b] @ W_lift.T + b_lift
        for s in range(S):
            nb, rw = nbr[b, s], row[b, s]
            a = np.concatenate([x_coord[b][nb], lat[rw]], axis=-1)
            h = gelu(a @ W1.T + b1)
            h = gelu(h @ W2.T + b2)
            k = (h @ W3.T + b3) * pn[nb]
            sums = np.zeros((M, COUT), np.float32)
            cnts = np.zeros((M,), np.float32)
            np.add.at(sums, rw, k)
            np.add.at(cnts, rw, 1.0)
            out[b] += (sums / np.maximum(cnts, 1.0)[:, None]) * sw[:, s][:, None]
    return out


def kernel(x_coord, pndata, latent_tokens_coord, nbr_idx, row_idx,
           W_lift, b_lift, W1, b1, W2, b2, W3, b3, Ws1, bs1, Ws2, bs2):
    x_coord = np.asarray(x_coord, dtype=np.float32)
    pndata = np.asarray(pndata, dtype=np.float32)
    lat = np.asarray(latent_tokens_coord, dtype=np.float32)
    nbr = np.asarray(nbr_idx).astype(np.int32)
    row = np.asarray(row_idx).astype(np.int32)
    f32 = lambda a: np.asarray(a, dtype=np.float32)
    Wl, bl = f32(W_lift), f32(b_lift)
    W1f, b1f, W2f, b2f, W3f, b3f = map(f32, (W1, b1, W2, b2, W3, b3))
    sw = _softmax_weights(lat, f32(Ws1), f32(bs1), f32(Ws2), f32(bs2))

    # row-quarter boundaries per (b, s): rows are sorted along E
    bounds = np.empty((B, S, 5), np.int64)
    for b in range(B):
        for s in range(S):
            bounds[b, s] = np.searchsorted(row[b, s], [0, MQ, 2 * MQ, 3 * MQ, M])
    caps = bounds[:, :, 1:] - bounds[:, :, :-1]            # chunk lengths
    ecap = int(-(-int(caps.max()) // 1024) * 1024)         # pad to 1K multiple

    xb = np.empty((NDEV, N, CD), np.float32)
    pnb = np.empty((NDEV, N, CIN), np.float32)
    nbrs = np.zeros((NDEV, S, ecap), np.int32)
    rows_g = np.zeros((NDEV, S, ecap), np.int32)           # global (for lat)
    rows_l = np.full((NDEV, S, ecap), MQ, np.int32)        # local (for scatter)
    wcnt_q = np.empty((NDEV, S, MQ), np.float32)
    for d in range(NDEV):
        b, q = d // 4, d % 4
        xb[d] = x_coord[b]
        pnb[d] = pndata[b]
        for s in range(S):
            lo, hi = bounds[b, s, q], bounds[b, s, q + 1]
            n = hi - lo
            nbrs[d, s, :n] = nbr[b, s, lo:hi]
            rows_g[d, s, :n] = row[b, s, lo:hi]
            rl = row[b, s, lo:hi] - q * MQ
            rows_l[d, s, :n] = rl
            cnt = np.bincount(rl, minlength=MQ).astype(np.float32)
            wcnt_q[d, s] = sw[q * MQ:(q + 1) * MQ, s] / np.maximum(cnt, 1.0)

    try:
        ks = _stage_a(xb, pnb, nbrs, rows_g, lat, Wl, bl,
                      W1f, b1f, W2f, b2f, W3f, b3f)
        out_d = _stage_b(ks, rows_l, wcnt_q)
        out_q = np.asarray(jax.device_get(out_d))          # [8, MQ, COUT]
        out = np.empty((B, M, COUT), np.float32)
        for d in range(NDEV):
            b, q = d // 4, d % 4
            out[b, q * MQ:(q + 1) * MQ] = out_q[d]
        return out
    except Exception:
        return _numpy_fallback(x_coord, pndata, lat, nbr, row, Wl, bl,
                               W1f, b1f, W2f, b2f, W3f, b3f, sw)



# revision 2
# speedup vs baseline: 1.0718x; 1.0718x over previous
"""MAGNO encoder across 8 axon-tunneled Trainium2 NeuronCores.

The axon wire is the bottleneck (~40MB/s H2D, ~20MB/s D2H, ~30ms/transfer
fixed), so the whole design minimizes bytes on the wire and round trips:

- ONE sharded H2D of a bit-packed payload (~6.5MB total):
    coords as u16 fixed-point, pndata as f16, nbr_idx as 16+1 bits
    (lo u16 + packed hi bit), sorted row_idx as 2-bit deltas, softmax
    scale weights as f16, MLP weights as f32.
- ONE fused shard_map jit does everything on device: index decode,
  table all_gather (ICI is ~free), gathers, kernel MLP, and a
  matmul-blocked cumsum + boundary-gather instead of scatter-add
  (row_idx is sorted, so segment_sum = diff of cumsum at boundaries).
- ONE D2H of the f16 output (~2.1MB).

Sharding: device d = (batch d//4, latent-row quarter d%4). row_idx is
sorted so each quarter's edges are a contiguous range; every edge lands
on exactly one device and per-device outputs are disjoint.
"""

import numpy as np
import jax
import jax.numpy as jnp
from jax.sharding import Mesh, NamedSharding, PartitionSpec as P
from jax.experimental.shard_map import shard_map
from functools import partial

B, N, M, S, E = 2, 100000, 16384, 3, 262144
CD, CIN, COUT, HID = 2, 3, 32, 64
NDEV = 8
MQ = M // 4                      # latent rows per device
NQ = N // 4                      # xpn table rows shipped per device
LQ = M // NDEV                   # lat table rows shipped per device
ECAP = 69632                     # padded edges per (device, scale); 544*128
NBLK = ECAP // 128

# payload section sizes (bytes), per device
_SZ_X = NQ * 2 * 2
_SZ_PN = NQ * 3 * 2
_SZ_LAT = LQ * 2 * 2
_SZ_NLO = S * ECAP * 2
_SZ_NHI = S * (ECAP // 8)
_SZ_ROW = S * (ECAP // 4)
_SZ_SW = S * MQ * 2
_NWTS = (HID * 4 + HID) + (HID * HID + HID) + (COUT * HID + COUT) \
    + (COUT * 4 + COUT) + S  # MLP weights (Wl padded to 4 cols) + row bases
_SZ_WTS = _NWTS * 4
_OFFS = np.cumsum([0, _SZ_X, _SZ_PN, _SZ_LAT, _SZ_NLO, _SZ_NHI, _SZ_ROW,
                   _SZ_SW, _SZ_WTS])
L = int(-(-_OFFS[-1] // 128) * 128)

_mesh = None
_run = None


def _u16(buf):
    return jax.lax.bitcast_convert_type(buf.reshape(-1, 2), jnp.uint16)


def _f32(buf):
    return jax.lax.bitcast_convert_type(buf.reshape(-1, 4), jnp.float32)


def _blocked_cumsum(x, tril_p, tril_b):
    """Inclusive cumsum along axis 0 of [ECAP, C] via matmuls."""
    C = x.shape[1]
    xb = x.reshape(NBLK, 128, C)
    within = jnp.einsum('pq,nqc->npc', tril_p, xb,
                        preferred_element_type=jnp.float32)
    totals = within[:, -1, :]                              # [NBLK, C]
    offs = jnp.einsum('bB,Bc->bc', tril_b, totals,
                      preferred_element_type=jnp.float32)  # exclusive
    return (within + offs[:, None, :]).reshape(ECAP, C)


def _body(payload):
    payload = payload.reshape(L)
    d = jax.lax.axis_index('d')

    o = [int(v) for v in _OFFS]
    x_q = _u16(payload[o[0]:o[0] + _SZ_X]).reshape(NQ, 2)
    pn_q = jax.lax.bitcast_convert_type(
        payload[o[1]:o[1] + _SZ_PN].reshape(-1, 2), jnp.float16
    ).reshape(NQ, 3)
    lat_q = _u16(payload[o[2]:o[2] + _SZ_LAT]).reshape(LQ, 2)
    nlo = _u16(payload[o[3]:o[3] + _SZ_NLO]).reshape(S, ECAP)
    nhi = payload[o[4]:o[4] + _SZ_NHI].reshape(S, ECAP // 8)
    rowd = payload[o[5]:o[5] + _SZ_ROW].reshape(S, ECAP // 4)
    swq = jax.lax.bitcast_convert_type(
        payload[o[6]:o[6] + _SZ_SW].reshape(-1, 2), jnp.float16
    ).reshape(S, MQ)
    wts = _f32(payload[o[7]:o[7] + _SZ_WTS])

    i = 0

    def take(n, shape):
        nonlocal i
        v = wts[i:i + n].reshape(shape)
        i += n
        return v

    W1 = take(HID * 4, (HID, 4))
    b1 = take(HID, (HID,))
    W2 = take(HID * HID, (HID, HID))
    b2 = take(HID, (HID,))
    W3 = take(COUT * HID, (COUT, HID))
    b3 = take(COUT, (COUT,))
    Wl4 = take(COUT * 4, (COUT, 4))
    bl = take(COUT, (COUT,))
    base = take(S, (S,))                                   # f32 row bases

    # replicate tables over the cores via ICI (cheap) instead of the wire.
    # Decode to f32 BEFORE any all_gather / gather / slice: u16 tensors in
    # gathers or dynamic slices crash the neuron compiler. Batch selection
    # is arithmetic blending, not dynamic_slice, for the same reason.
    isb = (d >= 4).astype(jnp.float32)                     # scalar 0/1
    x_all = jax.lax.all_gather(
        x_q.astype(jnp.float32) * (1.0 / 65535.0), 'd',
        tiled=True).reshape(2, N, 2)
    x_tab = x_all[0] + isb * (x_all[1] - x_all[0])         # [N, 2] f32
    pn4_q = jnp.concatenate([
        pn_q.astype(jnp.float32), jnp.zeros((NQ, 1), jnp.float32)], axis=1)
    pn_all = jax.lax.all_gather(
        pn4_q, 'd', tiled=True).reshape(2, N, 4)
    pn_tab = pn_all[0] + isb * (pn_all[1] - pn_all[0])     # [N, 4] f32
    lat_tab = jax.lax.all_gather(
        lat_q.astype(jnp.float32) * (1.0 / 65535.0), 'd', tiled=True)

    tril_p = jnp.tril(jnp.ones((128, 128), jnp.float32))
    tril_b = jnp.tril(jnp.ones((NBLK, NBLK), jnp.float32), k=-1)
    shifts8 = jnp.arange(8, dtype=jnp.uint8)
    shifts4 = jnp.arange(4, dtype=jnp.uint8) * 2

    out = jnp.zeros((MQ, COUT), jnp.float32)
    for s in range(S):
        hi = ((nhi[s][:, None] >> shifts8) & 1).reshape(ECAP)
        nbr = nlo[s].astype(jnp.int32) + (hi.astype(jnp.int32) << 16)
        deltas = ((rowd[s][:, None] >> shifts4) & 3).reshape(ECAP)
        row_f = _blocked_cumsum(
            deltas.astype(jnp.float32)[:, None], tril_p, tril_b)[:, 0]
        row = (row_f + base[s]).astype(jnp.int32)          # global row ids

        xg = x_tab[nbr]                                    # [ECAP, 2] f32
        pg = pn_tab[nbr]                                   # [ECAP, 4] f32
        # pad-ramp rows run past M-1 (so searchsorted excludes pads);
        # clamp for the gather only
        lg = lat_tab[jnp.minimum(row, M - 1)]              # [ECAP, 2] f32
        coords = jnp.concatenate([xg, lg], axis=1)         # [ECAP, 4]

        h = jax.nn.gelu(coords @ W1.T + b1)
        h = jax.nn.gelu(h @ W2.T + b2)
        k = (h @ W3.T + b3) * (pg @ Wl4.T + bl)            # [ECAP, COUT]

        cs = _blocked_cumsum(k, tril_p, tril_b)
        s0 = jnp.concatenate([jnp.zeros((1, COUT), jnp.float32), cs])

        needles = base[s].astype(jnp.int32) + jnp.arange(MQ + 1)
        bounds = jnp.searchsorted(row, needles, side='left')  # [MQ+1]
        seg = s0[bounds[1:]] - s0[bounds[:-1]]             # [MQ, COUT]
        cnt = (bounds[1:] - bounds[:-1]).astype(jnp.float32)
        w = swq[s].astype(jnp.float32) / jnp.maximum(cnt, 1.0)
        out = out + seg * w[:, None]

    return out.astype(jnp.float16)[None]                   # [1, MQ, COUT]


def _get_run():
    global _mesh, _run
    if _run is None:
        devs = jax.devices()[:NDEV]
        _mesh = Mesh(np.array(devs), ('d',))
        fn = shard_map(_body, mesh=_mesh,
                       in_specs=P('d', None),
                       out_specs=P('d', None, None))
        _run = jax.jit(fn)
    return _run


def _softmax_weights(lat, Ws1, bs1, Ws2, bs2):
    h = np.maximum(lat @ Ws1.T + bs1, 0.0) @ Ws2.T + bs2   # [M, S]
    h -= h.max(axis=-1, keepdims=True)
    e = np.exp(h)
    return e / e.sum(axis=-1, keepdims=True)


def _pack(x_coord, pndata, lat, nbr, row, Wl, bl, W1, b1, W2, b2, W3, b3, sw):
    """Build the [NDEV, L] uint8 payload. Returns None if the packing
    assumptions (quarter size <= ECAP, row deltas <= 3) don't hold."""
    payload = np.zeros((NDEV, L), np.uint8)

    xq = np.rint(x_coord * 65535.0).astype(np.uint16)      # [B, N, 2]
    pnq = pndata.astype(np.float16).view(np.uint16)        # [B, N, 3]
    latq = np.rint(lat * 65535.0).astype(np.uint16)        # [M, 2]

    o = [int(v) for v in _OFFS]
    wts_base = np.empty((NDEV, _NWTS), np.float32)
    ok = True
    for d in range(NDEV):
        b, q = d // 4, d % 4
        pl = payload[d]
        pl[o[0]:o[0] + _SZ_X] = np.frombuffer(
            xq[b, q * NQ:(q + 1) * NQ].tobytes(), np.uint8)
        pl[o[1]:o[1] + _SZ_PN] = np.frombuffer(
            pnq[b, q * NQ:(q + 1) * NQ].tobytes(), np.uint8)
        pl[o[2]:o[2] + _SZ_LAT] = np.frombuffer(
            latq[d * LQ:(d + 1) * LQ].tobytes(), np.uint8)

        nlo = np.zeros((S, ECAP), np.uint16)
        nhi = np.zeros((S, ECAP), np.uint8)
        rdel = np.zeros((S, ECAP), np.uint8)
        for s in range(S):
            r = row[b, s]
            lo_i = int(np.searchsorted(r, q * MQ))
            hi_i = int(np.searchsorted(r, (q + 1) * MQ))
            n = hi_i - lo_i
            if n > ECAP:
                ok = False
                break
            rr = r[lo_i:hi_i]
            nn = nbr[b, s, lo_i:hi_i]
            nlo[s, :n] = (nn & 0xFFFF).astype(np.uint16)
            nhi[s, :n] = (nn >> 16).astype(np.uint8)
            if n > 0:
                base_row = int(rr[0])
                d_i = np.diff(rr, prepend=rr[0])
            else:
                base_row = q * MQ
                d_i = np.empty((0,), np.int64)
            if n > 0 and d_i.max(initial=0) > 3:
                ok = False
                break
            rdel[s, :n] = d_i.astype(np.uint8)
            # pad ramp: walk rows past the end of this quarter so the
            # device-side searchsorted never counts pad edges
            last = base_row + int(d_i.sum())
            need = (q + 1) * MQ - last
            if need > 0:
                npad = ECAP - n
                nfull = min(need // 3, npad)
                rdel[s, n:n + nfull] = 3
                rem = need - 3 * nfull
                if rem > 0 and n + nfull < ECAP:
                    rdel[s, n + nfull] = rem
                elif rem > 0:
                    ok = False
                    break
            wts_base[d, _NWTS - S + s] = float(base_row)
        if not ok:
            break

        pl[o[3]:o[3] + _SZ_NLO] = np.frombuffer(nlo.tobytes(), np.uint8)
        pl[o[4]:o[4] + _SZ_NHI] = np.packbits(
            nhi.reshape(-1), bitorder='little')
        two = rdel.reshape(S, ECAP // 4, 4)
        pl[o[5]:o[5] + _SZ_ROW] = (
            two[:, :, 0] | (two[:, :, 1] << 2) | (two[:, :, 2] << 4)
            | (two[:, :, 3] << 6)).reshape(-1)
        pl[o[6]:o[6] + _SZ_SW] = np.frombuffer(
            sw[q * MQ:(q + 1) * MQ].T.astype(np.float16).tobytes(), np.uint8)

    if not ok:
        return None
    Wl4 = np.concatenate([Wl, np.zeros((COUT, 1), np.float32)], axis=1)
    flat = np.concatenate([W1.ravel(), b1.ravel(), W2.ravel(), b2.ravel(),
                           W3.ravel(), b3.ravel(), Wl4.ravel(), bl.ravel()]
                          ).astype(np.float32)
    wts_base[:, :flat.size] = flat
    payload[:, o[7]:o[7] + _SZ_WTS] = np.frombuffer(
        wts_base.tobytes(), np.uint8).reshape(NDEV, _SZ_WTS)
    return payload


def _numpy_fallback(x_coord, pndata, lat, nbr, row, W_lift, b_lift,
                    W1, b1, W2, b2, W3, b3, sw):
    def gelu(x):
        return 0.5 * x * (1.0 + np.tanh(
            np.sqrt(2 / np.pi) * (x + 0.044715 * x ** 3)))
    out = np.zeros((B, M, COUT), np.float32)
    for b in range(B):
        pn = pndata[b] @ W_lift.T + b_lift
        for s in range(S):
            nb, rw = nbr[b, s], row[b, s]
            a = np.concatenate([x_coord[b][nb], lat[rw]], axis=-1)
            h = gelu(a @ W1.T + b1)
            h = gelu(h @ W2.T + b2)
            k = (h @ W3.T + b3) * pn[nb]
            sums = np.zeros((M, COUT), np.float32)
            cnts = np.zeros((M,), np.float32)
            np.add.at(sums, rw, k)
            np.add.at(cnts, rw, 1.0)
            out[b] += (sums / np.maximum(cnts, 1.0)[:, None]) \
                * sw[:, s][:, None]
    return out


def kernel(x_coord, pndata, latent_tokens_coord, nbr_idx, row_idx,
           W_lift, b_lift, W1, b1, W2, b2, W3, b3, Ws1, bs1, Ws2, bs2):
    x_coord = np.asarray(x_coord, dtype=np.float32)
    pndata = np.asarray(pndata, dtype=np.float32)
    lat = np.asarray(latent_tokens_coord, dtype=np.float32)
    nbr = np.asarray(nbr_idx).astype(np.int64)
    row = np.asarray(row_idx).astype(np.int64)
    f32 = lambda a: np.asarray(a, dtype=np.float32)
    Wl, bl = f32(W_lift), f32(b_lift)
    W1f, b1f, W2f, b2f, W3f, b3f = map(f32, (W1, b1, W2, b2, W3, b3))
    sw = _softmax_weights(lat, f32(Ws1), f32(bs1), f32(Ws2), f32(bs2))

    payload = _pack(x_coord, pndata, lat, nbr, row, Wl, bl,
                    W1f, b1f, W2f, b2f, W3f, b3f, sw)
    if payload is None:
        return _numpy_fallback(x_coord, pndata, lat, nbr, row, Wl, bl,
                               W1f, b1f, W2f, b2f, W3f, b3f, sw)
    try:
        run = _get_run()
        sharding = NamedSharding(_mesh, P('d', None))
        pd = jax.device_put(payload, sharding)
        out_d = run(pd)                                    # [8, MQ, COUT] f16
        out_q = np.asarray(out_d).astype(np.float32)
        return out_q.reshape(B, M, COUT)
    except Exception:
        import os
        if os.environ.get("K_DEBUG"):
            raise
        return _numpy_fallback(x_coord, pndata, lat, nbr, row, Wl, bl,
                               W1f, b1f, W2f, b2f, W3f, b3f, sw)


# revision 4
# speedup vs baseline: 1.5206x; 1.4188x over previous
"""MAGNO encoder across 8 axon-tunneled Trainium2 NeuronCores, v3.

The axon wire is the bottleneck (~40MB/s H2D, ~20MB/s D2H, ~30ms fixed
per transfer, but full duplex and async-pipelined), so:

- All inputs bit-packed (~3.4MB per batch): coords u16 fixed-point,
  pndata f16, nbr_idx as 16+1 bits, sorted row_idx as 2-bit deltas,
  segment boundaries u16, softmax/count weights f16.
- TWO pipelined calls of ONE compiled shard_map jit (one per batch,
  all 8 cores each): batch 0's output D2H overlaps batch 1's input H2D
  and compute. Tables travel the wire sharded once and are replicated
  on-device over ICI via all_gather (~free).
- Segment-sum of the sorted edges = diff of a matmul-blocked cumsum at
  host-precomputed boundaries (no scatter, no device searchsorted).

Device d handles latent-row octant [2048*d, 2048*(d+1)) of the call's
batch; every edge lands on exactly one device, outputs are disjoint.
"""

import numpy as np
import jax
import jax.numpy as jnp
from jax.sharding import Mesh, NamedSharding, PartitionSpec as P
from jax.experimental.shard_map import shard_map

B, N, M, S, E = 2, 100000, 16384, 3, 262144
CD, CIN, COUT, HID = 2, 3, 32, 64
NDEV = 8
MO = M // NDEV                   # latent rows per device (octant)
NQ = N // NDEV                   # x/pn table rows shipped per device
LQ = M // NDEV                   # lat table rows shipped per device
ECAP = 34304                     # padded edges per (device, scale); 268*128
NBLK = ECAP // 128

_SZ_X = NQ * 2 * 2
_SZ_PN = NQ * 3 * 2
_SZ_LAT = LQ * 2 * 2
_SZ_NLO = S * ECAP * 2
_SZ_NHI = S * (ECAP // 8)
_SZ_ROW = S * (ECAP // 4)
_SZ_BND = S * (MO + 1) * 2       # u16 edge positions of row boundaries
_SZ_WC = S * MO * 2              # f16 softmax_weight / max(count,1)
_NWTS = (HID * 4 + HID) + (HID * HID + HID) + (COUT * HID + COUT) \
    + (COUT * 4 + COUT) + S      # MLP weights (Wl padded to 4) + row bases
_SZ_WTS = _NWTS * 4
_OFFS = np.cumsum([0, _SZ_X, _SZ_PN, _SZ_LAT, _SZ_NLO, _SZ_NHI, _SZ_ROW,
                   _SZ_BND, _SZ_WC, _SZ_WTS])
L = int(-(-_OFFS[-1] // 128) * 128)

_mesh = None
_run = None
_sharding = None


def _u16(buf):
    return jax.lax.bitcast_convert_type(buf.reshape(-1, 2), jnp.uint16)


def _blocked_cumsum(x, tril_p, tril_b):
    """Inclusive cumsum along axis 0 of [ECAP, C] via matmuls."""
    C = x.shape[1]
    xb = x.reshape(NBLK, 128, C)
    within = jnp.einsum('pq,nqc->npc', tril_p, xb,
                        preferred_element_type=jnp.float32)
    totals = within[:, -1, :]                              # [NBLK, C]
    offs = jnp.einsum('bB,Bc->bc', tril_b, totals,
                      preferred_element_type=jnp.float32)  # exclusive
    return (within + offs[:, None, :]).reshape(ECAP, C)


def _body(payload):
    payload = payload.reshape(L)

    o = [int(v) for v in _OFFS]
    x_q = _u16(payload[o[0]:o[0] + _SZ_X]).reshape(NQ, 2)
    pn_q = jax.lax.bitcast_convert_type(
        payload[o[1]:o[1] + _SZ_PN].reshape(-1, 2), jnp.float16
    ).reshape(NQ, 3)
    lat_q = _u16(payload[o[2]:o[2] + _SZ_LAT]).reshape(LQ, 2)
    nlo = _u16(payload[o[3]:o[3] + _SZ_NLO]).reshape(S, ECAP)
    nhi = payload[o[4]:o[4] + _SZ_NHI].reshape(S, ECAP // 8)
    rowd = payload[o[5]:o[5] + _SZ_ROW].reshape(S, ECAP // 4)
    bnd = _u16(payload[o[6]:o[6] + _SZ_BND]).reshape(S, MO + 1)
    wc = jax.lax.bitcast_convert_type(
        payload[o[7]:o[7] + _SZ_WC].reshape(-1, 2), jnp.float16
    ).reshape(S, MO)
    wts = jax.lax.bitcast_convert_type(
        payload[o[8]:o[8] + _SZ_WTS].reshape(-1, 4), jnp.float32)

    i = 0

    def take(n, shape):
        nonlocal i
        v = wts[i:i + n].reshape(shape)
        i += n
        return v

    W1 = take(HID * 4, (HID, 4))
    b1 = take(HID, (HID,))
    W2 = take(HID * HID, (HID, HID))
    b2 = take(HID, (HID,))
    W3 = take(COUT * HID, (COUT, HID))
    b3 = take(COUT, (COUT,))
    Wl4 = take(COUT * 4, (COUT, 4))
    bl = take(COUT, (COUT,))
    base = take(S, (S,))                                   # f32 row bases

    # replicate tables on-device via ICI instead of the wire; decode to
    # f32 BEFORE all_gather/gather (u16 gathers crash the neuron compiler)
    x_tab = jax.lax.all_gather(
        x_q.astype(jnp.float32) * (1.0 / 65535.0), 'd', tiled=True)
    pn4_q = jnp.concatenate([
        pn_q.astype(jnp.float32), jnp.zeros((NQ, 1), jnp.float32)], axis=1)
    pn_tab = jax.lax.all_gather(pn4_q, 'd', tiled=True)    # [N, 4] f32
    lat_tab = jax.lax.all_gather(
        lat_q.astype(jnp.float32) * (1.0 / 65535.0), 'd', tiled=True)

    tril_p = jnp.tril(jnp.ones((128, 128), jnp.float32))
    tril_b = jnp.tril(jnp.ones((NBLK, NBLK), jnp.float32), k=-1)
    shifts8 = jnp.arange(8, dtype=jnp.uint8)
    shifts4 = jnp.arange(4, dtype=jnp.uint8) * 2

    out = jnp.zeros((MO, COUT), jnp.float32)
    for s in range(S):
        hi = ((nhi[s][:, None] >> shifts8) & 1).reshape(ECAP)
        nbr = nlo[s].astype(jnp.int32) + (hi.astype(jnp.int32) << 16)
        deltas = ((rowd[s][:, None] >> shifts4) & 3).reshape(ECAP)
        row_f = _blocked_cumsum(
            deltas.astype(jnp.float32)[:, None], tril_p, tril_b)[:, 0]
        row = jnp.minimum(
            (row_f + base[s]).astype(jnp.int32), M - 1)    # global row ids

        xg = x_tab[nbr]                                    # [ECAP, 2] f32
        pg = pn_tab[nbr]                                   # [ECAP, 4] f32
        lg = lat_tab[row]                                  # [ECAP, 2] f32
        coords = jnp.concatenate([xg, lg], axis=1)         # [ECAP, 4]

        h = jax.nn.gelu(coords @ W1.T + b1)
        h = jax.nn.gelu(h @ W2.T + b2)
        k = (h @ W3.T + b3) * (pg @ Wl4.T + bl)            # [ECAP, COUT]

        cs = _blocked_cumsum(k, tril_p, tril_b)
        s0 = jnp.concatenate([jnp.zeros((1, COUT), jnp.float32), cs])

        bounds = bnd[s].astype(jnp.int32)                  # [MO+1]
        seg = s0[bounds[1:]] - s0[bounds[:-1]]             # [MO, COUT]
        out = out + seg * wc[s].astype(jnp.float32)[:, None]

    return out.astype(jnp.float16)[None]                   # [1, MO, COUT]


def _get_run():
    global _mesh, _run, _sharding
    if _run is None:
        devs = jax.devices()[:NDEV]
        _mesh = Mesh(np.array(devs), ('d',))
        fn = shard_map(_body, mesh=_mesh,
                       in_specs=P('d', None),
                       out_specs=P('d', None, None))
        _run = jax.jit(fn)
        _sharding = NamedSharding(_mesh, P('d', None))
    return _run


def _softmax_weights(lat, Ws1, bs1, Ws2, bs2):
    h = np.maximum(lat @ Ws1.T + bs1, 0.0) @ Ws2.T + bs2   # [M, S]
    h -= h.max(axis=-1, keepdims=True)
    e = np.exp(h)
    return e / e.sum(axis=-1, keepdims=True)


def _pack_batch(b, xq, pnq, latq, nbr, row, sw, wflat):
    """Build the [NDEV, L] uint8 payload for one batch.
    Returns None if packing assumptions fail (octant > ECAP, delta > 3)."""
    payload = np.zeros((NDEV, L), np.uint8)
    o = [int(v) for v in _OFFS]
    for d in range(NDEV):
        pl = payload[d]
        pl[o[0]:o[0] + _SZ_X] = np.frombuffer(
            xq[b, d * NQ:(d + 1) * NQ].tobytes(), np.uint8)
        pl[o[1]:o[1] + _SZ_PN] = np.frombuffer(
            pnq[b, d * NQ:(d + 1) * NQ].tobytes(), np.uint8)
        pl[o[2]:o[2] + _SZ_LAT] = np.frombuffer(
            latq[d * LQ:(d + 1) * LQ].tobytes(), np.uint8)

        nlo = np.zeros((S, ECAP), np.uint16)
        nhi = np.zeros((S, ECAP), np.uint8)
        rdel = np.zeros((S, ECAP), np.uint8)
        bnds = np.zeros((S, MO + 1), np.uint16)
        wcs = np.zeros((S, MO), np.float16)
        wbase = np.zeros((S,), np.float32)
        for s in range(S):
            r = row[b, s]
            lo_i = int(np.searchsorted(r, d * MO))
            hi_i = int(np.searchsorted(r, (d + 1) * MO))
            n = hi_i - lo_i
            if n > ECAP:
                return None
            rr = r[lo_i:hi_i]
            nn = nbr[b, s, lo_i:hi_i]
            nlo[s, :n] = (nn & 0xFFFF).astype(np.uint16)
            nhi[s, :n] = (nn >> 16).astype(np.uint8)
            if n > 0:
                base_row = int(rr[0])
                d_i = np.diff(rr, prepend=rr[0])
                if d_i.max(initial=0) > 3:
                    return None
                rdel[s, :n] = d_i.astype(np.uint8)
            else:
                base_row = d * MO
            wbase[s] = float(base_row)
            bb = np.searchsorted(
                rr, np.arange(d * MO, (d + 1) * MO + 1)).astype(np.uint16)
            bnds[s] = bb
            cnt = (bb[1:].astype(np.int32) - bb[:-1].astype(np.int32))
            wcs[s] = (sw[d * MO:(d + 1) * MO, s]
                      / np.maximum(cnt, 1)).astype(np.float16)

        pl[o[3]:o[3] + _SZ_NLO] = np.frombuffer(nlo.tobytes(), np.uint8)
        pl[o[4]:o[4] + _SZ_NHI] = np.packbits(
            nhi.reshape(-1), bitorder='little')
        two = rdel.reshape(S, ECAP // 4, 4)
        pl[o[5]:o[5] + _SZ_ROW] = (
            two[:, :, 0] | (two[:, :, 1] << 2) | (two[:, :, 2] << 4)
            | (two[:, :, 3] << 6)).reshape(-1)
        pl[o[6]:o[6] + _SZ_BND] = np.frombuffer(bnds.tobytes(), np.uint8)
        pl[o[7]:o[7] + _SZ_WC] = np.frombuffer(wcs.tobytes(), np.uint8)
        wb = np.concatenate([wflat, wbase]).astype(np.float32)
        pl[o[8]:o[8] + _SZ_WTS] = np.frombuffer(wb.tobytes(), np.uint8)
    return payload


def _numpy_fallback(x_coord, pndata, lat, nbr, row, W_lift, b_lift,
                    W1, b1, W2, b2, W3, b3, sw):
    def gelu(x):
        return 0.5 * x * (1.0 + np.tanh(
            np.sqrt(2 / np.pi) * (x + 0.044715 * x ** 3)))
    out = np.zeros((B, M, COUT), np.float32)
    for b in range(B):
        pn = pndata[b] @ W_lift.T + b_lift
        for s in range(S):
            nb, rw = nbr[b, s], row[b, s]
            a = np.concatenate([x_coord[b][nb], lat[rw]], axis=-1)
            h = gelu(a @ W1.T + b1)
            h = gelu(h @ W2.T + b2)
            k = (h @ W3.T + b3) * pn[nb]
            sums = np.zeros((M, COUT), np.float32)
            cnts = np.zeros((M,), np.float32)
            np.add.at(sums, rw, k)
            np.add.at(cnts, rw, 1.0)
            out[b] += (sums / np.maximum(cnts, 1.0)[:, None]) \
                * sw[:, s][:, None]
    return out


def kernel(x_coord, pndata, latent_tokens_coord, nbr_idx, row_idx,
           W_lift, b_lift, W1, b1, W2, b2, W3, b3, Ws1, bs1, Ws2, bs2):
    x_coord = np.asarray(x_coord, dtype=np.float32)
    pndata = np.asarray(pndata, dtype=np.float32)
    lat = np.asarray(latent_tokens_coord, dtype=np.float32)
    nbr = np.asarray(nbr_idx).astype(np.int64)
    row = np.asarray(row_idx).astype(np.int64)
    f32 = lambda a: np.asarray(a, dtype=np.float32)
    Wl, bl = f32(W_lift), f32(b_lift)
    W1f, b1f, W2f, b2f, W3f, b3f = map(f32, (W1, b1, W2, b2, W3, b3))
    sw = _softmax_weights(lat, f32(Ws1), f32(bs1), f32(Ws2), f32(bs2))

    xq = np.rint(np.clip(x_coord, 0.0, 1.0) * 65535.0).astype(np.uint16)
    pnq = pndata.astype(np.float16).view(np.uint16)
    latq = np.rint(np.clip(lat, 0.0, 1.0) * 65535.0).astype(np.uint16)
    Wl4 = np.concatenate([Wl, np.zeros((COUT, 1), np.float32)], axis=1)
    wflat = np.concatenate(
        [W1f.ravel(), b1f.ravel(), W2f.ravel(), b2f.ravel(),
         W3f.ravel(), b3f.ravel(), Wl4.ravel(), bl.ravel()]
    ).astype(np.float32)

    try:
        run = _get_run()
        # batch 0: pack -> async put -> async dispatch
        p0 = _pack_batch(0, xq, pnq, latq, nbr, row, sw, wflat)
        if p0 is None:
            raise _PackError()
        pd0 = jax.device_put(p0, _sharding)
        r0 = run(pd0)
        # batch 1 packs while batch 0 streams; put/dispatch pipeline behind
        p1 = _pack_batch(1, xq, pnq, latq, nbr, row, sw, wflat)
        if p1 is None:
            raise _PackError()
        pd1 = jax.device_put(p1, _sharding)
        r1 = run(pd1)
        try:
            r0.copy_to_host_async()
            r1.copy_to_host_async()
        except Exception:
            pass
        out = np.empty((B, M, COUT), np.float32)
        out[0] = np.asarray(r0).reshape(M, COUT).astype(np.float32)
        out[1] = np.asarray(r1).reshape(M, COUT).astype(np.float32)
        return out
    except Exception:
        import os
        if os.environ.get("K_DEBUG"):
            raise
        return _numpy_fallback(x_coord, pndata, lat, nbr, row, Wl, bl,
                               W1f, b1f, W2f, b2f, W3f, b3f, sw)


class _PackError(Exception):
    pass
